# revision 7
# baseline (speedup 1.0000x reference)
"""Trainium2 Bass kernel for nn_Decoder_TRANSFORMER_14791867367496.

The reference decoder is affine in the positions: each frame step is
    pos_{t+1} = pos_t @ M + (d_t[b] + g[b,j]),   M = I + W_pe @ W3  (3x3)
(with W_final = [W1; W2; W3] split along its 768 input rows), so the whole
60-step scan has a closed form

    out[b, j, :, t] = X[b, j, :] @ Q_t + r_t[b, :]

where X = initial_grid,
    Q_t = M^t + (W_pe @ W2) @ S_t,          S_t = sum_{k<t} M^k
    r_t[b] = h @ S_t + D_t[b],              D_t = sum_{s=1..t} d_s M^{t-s}
    d_t[b] = (emb_table[t] + z @ W_clip + b_clip) @ W1
    h      = b_pe @ (W2 + W3) + b_final

All of Q/r are tiny (3x3 / per-batch 3-vectors) and are computed on the host
in float64.  The device kernel is then a single affine map per point:
[points, 4] @ [4, 180] (3 features + a constant-1 row folded with the bias),
i.e. the kernel is purely output-bandwidth bound (94 MB of f32 output).

Sharding: data-parallel over batch — each of the 8 cores handles 4 batches
(16384 points).  Per core the kernel runs 128 matmuls of
[K=4, M=128pts] x [K=4, N=180], copies PSUM->SBUF staging (DVE/ACT), and
streams the output with 8 fully-linear 1.47 MB DMAs.
"""

import numpy as np

BS, NFRAMES, NJOINTS, NFEATS, LATENT, CLIP = 32, 60, 4096, 3, 256, 512
NCORES = 8
B_PER_CORE = BS // NCORES                  # 4
PTS = B_PER_CORE * NJOINTS                 # 16384 points per core
NTILES = PTS // 128                        # 128 point-tiles per core
GROUPS = 8                                 # output DMA groups
TPG = NTILES // GROUPS                     # 16 tiles per group
FC = NFEATS * NFRAMES                      # 180 output columns per point


def _precompute(z, W_pe, b_pe, W_clip, b_clip, emb_table, W_final, b_final):
    """Host-side f64 computation of the closed-form coefficients.

    Returns Q_all [3, 180] and r_all [32, 180], column layout c = f*60 + t
    (matching the [.., 3, 60] innermost layout of the output)."""
    f64 = np.float64
    W_pe64 = np.asarray(W_pe, f64)
    W_fin = np.asarray(W_final, f64)
    W1, W2, W3 = W_fin[:LATENT], W_fin[LATENT:2 * LATENT], W_fin[2 * LATENT:]
    M = np.eye(3) + W_pe64 @ W3
    Gm = W_pe64 @ W2
    b_pe64 = np.asarray(b_pe, f64)
    h = b_pe64 @ W2 + b_pe64 @ W3 + np.asarray(b_final, f64)
    z_proj = np.asarray(z, f64) @ np.asarray(W_clip, f64) + np.asarray(b_clip, f64)
    d = (np.asarray(emb_table, f64)[None, :, :] + z_proj[:, None, :]) @ W1  # [32,60,3]

    Q = np.zeros((NFRAMES, 3, 3))
    R = np.zeros((NFRAMES, BS, 3))
    Q[0] = np.eye(3)
    Mt = np.eye(3)
    S = np.zeros((3, 3))
    D = np.zeros((BS, 3))
    for t in range(1, NFRAMES):
        S = S + Mt
        Mt = Mt @ M
        D = D @ M + d[:, t, :]
        Q[t] = Mt + Gm @ S
        R[t] = h @ S + D
    Q_all = Q.transpose(1, 2, 0).reshape(3, FC)     # [k, f*60+t]
    r_all = R.transpose(1, 2, 0).reshape(BS, FC)    # [b, f*60+t]
    return Q_all.astype(np.float32), r_all.astype(np.float32)


def _build_bass():
    import concourse.mybir as mybir
    from concourse import bacc
    from concourse.bass import ts
    from concourse.tile import TileContext

    f32 = mybir.dt.float32
    # Bacc (not raw Bass): its compile() pass splits multi-waits into
    # EventSemaphore instructions — walrus allows only 1 wait per instruction.
    nc = bacc.Bacc(None, target_bir_lowering=False)
    xt = nc.dram_tensor("xt", [4, PTS], f32, kind="ExternalInput")
    rhs = nc.dram_tensor("rhs", [4, B_PER_CORE * FC], f32, kind="ExternalInput")
    out = nc.dram_tensor("out", [PTS, FC], f32, kind="ExternalOutput")
    # Point p of this core lives at tile i = p%... : points are assigned so
    # that group g's staging buffer [128, 16*180] maps to one fully linear
    # DRAM range: p = g*2048 + j*16 + w  (j = psum partition, w = tile-in-group).
    out_v = out[:].rearrange("(g j w) c -> g j (w c)", g=GROUPS, j=128, w=TPG)

    with TileContext(nc) as tc:
        with (
            tc.tile_pool(name="const", bufs=1) as cpool,
            tc.tile_pool(name="stage", bufs=3) as spool,
            tc.tile_pool(name="psum", bufs=7, space="PSUM") as ppool,
            tc.tile_pool(name="psum_d", bufs=1, space="PSUM") as dpool,
        ):
            rhs_sb = cpool.tile([4, B_PER_CORE * FC], f32, tag="rhs")
            nc.gpsimd.dma_start(out=rhs_sb[:], in_=rhs[:])
            xt_sb = []
            for g in range(GROUPS):
                xg = cpool.tile([4, TPG * 128], f32, tag=f"xt{g}")
                nc.gpsimd.dma_start(out=xg[:], in_=xt[:, ts(g, TPG * 128)])
                xt_sb.append(xg)
            dummy_ps = dpool.tile([1, 1], f32, tag="dps")

            # walrus allows only ONE sync wait per instruction, so the
            # pipeline is a strict chain (SWDGE-in -> PE -> DVE -> HWDGE-out)
            # and every buffer-reuse back-edge is absorbed by a 1-element
            # same-engine dummy op that carries the extra wait.
            for g in range(GROUPS):
                stage = spool.tile([128, TPG * FC], f32, tag="stage")
                # PE absorber: takes the chunk-DMA wait so real matmuls only
                # ever wait on the DVE psum-slot release.
                nc.tensor.matmul(
                    dummy_ps[:], xt_sb[g][:, 0:1], xt_sb[g][:, 0:1],
                    start=True, stop=True,
                )
                # DVE absorber: takes the stage-slot release wait (out-DMA of
                # an earlier group) so real copies only wait on PE.
                nc.vector.tensor_copy(out=stage[0:1, 0:1], in_=stage[0:1, 0:1])
                for w in range(TPG):
                    i = g * TPG + w
                    lb = i // (NTILES // B_PER_CORE)   # local batch of this tile
                    ps = ppool.tile([128, FC], f32, tag="ps")
                    nc.tensor.matmul(
                        ps[:],
                        xt_sb[g][:, ts(w, 128)],       # stationary [K=4, M=128]
                        rhs_sb[:, ts(lb, FC)],         # moving     [K=4, N=180]
                        start=True,
                        stop=True,
                    )
                    nc.vector.tensor_copy(out=stage[:, ts(w, FC)], in_=ps[:])
                nc.sync.dma_start(out=out_v[g], in_=stage[:])
    nc.finalize()   # Bacc: runs compile() — wait splitting + reg allocation
    return nc


_NC_CACHE = None
_LAST_RESULTS = None  # BassKernelResults of the most recent run (for profiling)


def kernel(z, mask, initial_grid, W_pe, b_pe, W_clip, b_clip, emb_table,
           W_final, b_final):
    global _NC_CACHE, _LAST_RESULTS
    from concourse import bass_utils

    Q_all, r_all = _precompute(z, W_pe, b_pe, W_clip, b_clip, emb_table,
                               W_final, b_final)
    X = np.ascontiguousarray(np.asarray(initial_grid), dtype=np.float32)

    in_maps = []
    for c in range(NCORES):
        Xc = X[B_PER_CORE * c:B_PER_CORE * (c + 1)].reshape(PTS, NFEATS)
        # xt column (g*16+w)*128 + j holds point p = g*2048 + j*16 + w
        X3 = Xc.reshape(GROUPS, 128, TPG, NFEATS)          # [g, j, w, k]
        X3 = X3.transpose(3, 0, 2, 1).reshape(NFEATS, PTS)  # [k, (g w j)]
        xt_host = np.empty((4, PTS), np.float32)
        xt_host[:NFEATS] = X3
        xt_host[NFEATS] = 1.0                               # bias row
        rhs_host = np.empty((4, B_PER_CORE * FC), np.float32)
        for lb in range(B_PER_CORE):
            rhs_host[:NFEATS, lb * FC:(lb + 1) * FC] = Q_all
            rhs_host[NFEATS, lb * FC:(lb + 1) * FC] = r_all[B_PER_CORE * c + lb]
        in_maps.append({"xt": np.ascontiguousarray(xt_host), "rhs": rhs_host})

    if _NC_CACHE is None:
        _NC_CACHE = _build_bass()
    res = bass_utils.run_bass_kernel_spmd(
        _NC_CACHE, in_maps, core_ids=list(range(NCORES))
    )
    _LAST_RESULTS = res

    out = np.empty((BS, NJOINTS, NFEATS, NFRAMES), np.float32)
    for c in range(NCORES):
        out[B_PER_CORE * c:B_PER_CORE * (c + 1)] = (
            res.results[c]["out"].reshape(B_PER_CORE, NJOINTS, NFEATS, NFRAMES)
        )
    return out


# revision 8
# speedup vs baseline: 1.8635x; 1.8635x over previous
"""Trainium2 Bass kernel for nn_Decoder_TRANSFORMER_14791867367496.

The reference decoder is affine in the positions: each frame step is
    pos_{t+1} = pos_t @ M + (d_t[b] + g[b,j]),   M = I + W_pe @ W3  (3x3)
(with W_final = [W1; W2; W3] split along its 768 input rows), so the whole
60-step scan has a closed form

    out[b, j, :, t] = X[b, j, :] @ Q_t + r_t[b, :]

where X = initial_grid,
    Q_t = M^t + (W_pe @ W2) @ S_t,          S_t = sum_{k<t} M^k
    r_t[b] = h @ S_t + D_t[b],              D_t = sum_{s=1..t} d_s M^{t-s}
    d_t[b] = (emb_table[t] + z @ W_clip + b_clip) @ W1
    h      = b_pe @ (W2 + W3) + b_final

All of Q/r are tiny (3x3 / per-batch 3-vectors) and are computed on the host
in float64.  The device kernel is then a single affine map per point
([3 feats + bias] -> 180 outputs) and is purely output-bandwidth bound
(94 MB of f32 output).

Precision trick: fp32 matmuls on trn2 run as two PE passes (~2x slower
streaming + 2x LDWEIGHTS).  Instead each operand is split into three bf16
chunks (x = x0+x1+x2, 8 mantissa bits each) and all cross terms with
a+b <= 2 are summed IN A SINGLE MATMUL by stacking them along the
contraction dim: rows [x0 x0 x1 x0 x1 x2] paired against
[q0 q1 q0 q2 q1 q0].  bf16 products are exact in fp32, so this matches
fp32 accuracy (~3e-8 measured) at bf16 speed.  Per point-pair-tile the
K-stack is 21 rows x 2 tiles = K=42, N=2*180=360 (block-diagonal rhs).

Sharding: data-parallel over batch — each of the 8 cores handles 4 batches
(16384 points = 128 point-tiles = 64 packed matmuls).  Output streams out
in 8 fully-linear 1.47 MB DMAs.
"""

import numpy as np

BS, NFRAMES, NJOINTS, NFEATS, LATENT, CLIP = 32, 60, 4096, 3, 256, 512
NCORES = 8
B_PER_CORE = BS // NCORES                  # 4
PTS = B_PER_CORE * NJOINTS                 # 16384 points per core
NTILES = PTS // 128                        # 128 point-tiles per core
GROUPS = 8                                 # output DMA groups
TPG = NTILES // GROUPS                     # 16 tiles per group
FC = NFEATS * NFRAMES                      # 180 output columns per point
KR = 21                                    # K-stack rows per tile (3*6 + 3 bias)
PAIR = 2                                   # tiles fused per matmul
MM_PER_G = TPG // PAIR                     # 8 matmuls per group
XCH = [0, 0, 1, 0, 1, 2]                   # x-chunk index per K row
QCH = [0, 1, 0, 2, 1, 0]                   # q-chunk index per K row


def _split3(a):
    """Split f32 array into three bf16 chunks whose sum reproduces ~24
    mantissa bits.  Returned as f32 arrays holding bf16-representable
    values."""
    import ml_dtypes
    bf = ml_dtypes.bfloat16
    a = np.asarray(a, np.float32)
    a0 = a.astype(bf).astype(np.float32)
    a1 = (a - a0).astype(bf).astype(np.float32)
    a2 = (a - a0 - a1).astype(bf).astype(np.float32)
    return a0, a1, a2


def _precompute(z, W_pe, b_pe, W_clip, b_clip, emb_table, W_final, b_final):
    """Host-side f64 computation of the closed-form coefficients.

    Returns Q_all [3, 180] and r_all [32, 180], column layout c = f*60 + t
    (matching the [.., 3, 60] innermost layout of the output)."""
    f64 = np.float64
    W_pe64 = np.asarray(W_pe, f64)
    W_fin = np.asarray(W_final, f64)
    W1, W2, W3 = W_fin[:LATENT], W_fin[LATENT:2 * LATENT], W_fin[2 * LATENT:]
    M = np.eye(3) + W_pe64 @ W3
    Gm = W_pe64 @ W2
    b_pe64 = np.asarray(b_pe, f64)
    h = b_pe64 @ W2 + b_pe64 @ W3 + np.asarray(b_final, f64)
    z_proj = np.asarray(z, f64) @ np.asarray(W_clip, f64) + np.asarray(b_clip, f64)
    d = (np.asarray(emb_table, f64)[None, :, :] + z_proj[:, None, :]) @ W1  # [32,60,3]

    Q = np.zeros((NFRAMES, 3, 3))
    R = np.zeros((NFRAMES, BS, 3))
    Q[0] = np.eye(3)
    Mt = np.eye(3)
    S = np.zeros((3, 3))
    D = np.zeros((BS, 3))
    for t in range(1, NFRAMES):
        S = S + Mt
        Mt = Mt @ M
        D = D @ M + d[:, t, :]
        Q[t] = Mt + Gm @ S
        R[t] = h @ S + D
    Q_all = Q.transpose(1, 2, 0).reshape(3, FC)     # [k, f*60+t]
    r_all = R.transpose(1, 2, 0).reshape(BS, FC)    # [b, f*60+t]
    return Q_all.astype(np.float32), r_all.astype(np.float32)


def _build_bass():
    import concourse.mybir as mybir
    from concourse import bacc
    from concourse.bass import ts
    from concourse.tile import TileContext

    f32 = mybir.dt.float32
    bf16 = mybir.dt.bfloat16
    nc = bacc.Bacc(None, target_bir_lowering=False)
    xt = nc.dram_tensor("xt", [PAIR * KR, NTILES // PAIR * 128], bf16,
                        kind="ExternalInput")
    rhs = nc.dram_tensor("rhs", [PAIR * KR, B_PER_CORE * PAIR * FC], bf16,
                         kind="ExternalInput")
    out = nc.dram_tensor("out", [PTS, FC], f32, kind="ExternalOutput")
    # Points are assigned so that group g's staging buffer [128, 16*180] maps
    # to one fully linear DRAM range: p = g*2048 + j*16 + w  (j = psum
    # partition, w = tile-in-group).
    out_v = out[:].rearrange("(g j w) c -> g j (w c)", g=GROUPS, j=128, w=TPG)

    with TileContext(nc) as tc:
        with (
            tc.tile_pool(name="const", bufs=1) as cpool,
            tc.tile_pool(name="stage", bufs=3) as spool,
            tc.tile_pool(name="psum", bufs=7, space="PSUM") as ppool,
            tc.tile_pool(name="psum_d", bufs=1, space="PSUM") as dpool,
        ):
            rhs_sb = cpool.tile([PAIR * KR, B_PER_CORE * PAIR * FC], bf16,
                                tag="rhs")
            nc.gpsimd.dma_start(out=rhs_sb[:], in_=rhs[:])
            xt_sb = []
            for g in range(GROUPS):
                xg = cpool.tile([PAIR * KR, MM_PER_G * 128], bf16, tag=f"xt{g}")
                nc.gpsimd.dma_start(out=xg[:], in_=xt[:, ts(g, MM_PER_G * 128)])
                xt_sb.append(xg)
            dummy_ps = dpool.tile([1, 1], f32, tag="dps")

            # Strict chain (SWDGE-in -> PE -> DVE/ACT -> HWDGE-out); 1-element
            # same-engine absorber ops take buffer-reuse back-edge waits so hot
            # instructions keep a single sync wait (cheap even though Bacc can
            # split multi-waits into EventSemaphores).
            for g in range(GROUPS):
                stage = spool.tile([128, TPG * FC], f32, tag="stage")
                nc.tensor.matmul(
                    dummy_ps[:], xt_sb[g][:, 0:1], xt_sb[g][:, 0:1],
                    start=True, stop=True,
                )
                if g % 2 == 0:
                    nc.vector.tensor_copy(out=stage[0:1, 0:1], in_=stage[0:1, 0:1])
                else:
                    nc.scalar.copy(out=stage[0:1, 0:1], in_=stage[0:1, 0:1])
                lb = g // 2                        # local batch of this group
                for sp in range(MM_PER_G):
                    ps = ppool.tile([128, PAIR * FC], f32, tag="ps")
                    nc.tensor.matmul(
                        ps[:],
                        xt_sb[g][:, ts(sp, 128)],      # stationary [42, 128]
                        rhs_sb[:, ts(lb, PAIR * FC)],  # moving     [42, 360]
                        start=True,
                        stop=True,
                    )
                    # PSUM -> SBUF staging; one engine per group (alternating
                    # DVE/ACT) keeps the out-DMA on a single semaphore.
                    if g % 2 == 0:
                        nc.vector.tensor_copy(out=stage[:, ts(sp, PAIR * FC)],
                                              in_=ps[:])
                    else:
                        nc.scalar.copy(out=stage[:, ts(sp, PAIR * FC)], in_=ps[:])
                nc.sync.dma_start(out=out_v[g], in_=stage[:])
    nc.finalize()   # Bacc: runs compile() — wait splitting + reg allocation
    return nc


_NC_CACHE = None
_LAST_RESULTS = None  # BassKernelResults of the most recent run (for profiling)


def kernel(z, mask, initial_grid, W_pe, b_pe, W_clip, b_clip, emb_table,
           W_final, b_final):
    global _NC_CACHE, _LAST_RESULTS
    import ml_dtypes
    from concourse import bass_utils

    bf = ml_dtypes.bfloat16
    Q_all, r_all = _precompute(z, W_pe, b_pe, W_clip, b_clip, emb_table,
                               W_final, b_final)
    Qs = _split3(Q_all)                                 # 3 x [3, 180]
    X = np.ascontiguousarray(np.asarray(initial_grid), dtype=np.float32)

    in_maps = []
    for c in range(NCORES):
        Xc = X[B_PER_CORE * c:B_PER_CORE * (c + 1)].reshape(PTS, NFEATS)
        # point p = g*2048 + j*16 + w lives at tile (g, w), psum partition j
        X4 = Xc.reshape(GROUPS, 128, TPG, NFEATS).transpose(3, 0, 2, 1)
        ch = _split3(X4)                                # 3 x [3, 8, 16, 128]
        A = np.empty((GROUPS, TPG, KR, 128), np.float32)
        for k in range(NFEATS):
            for m in range(6):
                A[:, :, 6 * k + m, :] = ch[XCH[m]][k]
        A[:, :, 18:21, :] = 1.0                         # bias rows
        # matmul s covers tiles (2*(s%8), 2*(s%8)+1) of group s//8;
        # stationary rows 21a.. hold tile a of the pair
        xt_host = (A.reshape(GROUPS, MM_PER_G, PAIR, KR, 128)
                   .transpose(2, 3, 0, 1, 4)
                   .reshape(PAIR * KR, NTILES // PAIR * 128)).astype(bf)

        rhs_host = np.zeros((PAIR * KR, B_PER_CORE * PAIR * FC), np.float32)
        for lb in range(B_PER_CORE):
            rs = _split3(r_all[B_PER_CORE * c + lb])    # 3 x [180]
            R = np.empty((KR, FC), np.float32)
            for k in range(NFEATS):
                for m in range(6):
                    R[6 * k + m] = Qs[QCH[m]][k]
            R[18:21] = np.stack(rs)
            for a in range(PAIR):                       # block-diagonal
                rhs_host[KR * a:KR * (a + 1),
                         lb * PAIR * FC + FC * a: lb * PAIR * FC + FC * (a + 1)] = R
        in_maps.append({"xt": np.ascontiguousarray(xt_host),
                        "rhs": rhs_host.astype(bf)})

    if _NC_CACHE is None:
        _NC_CACHE = _build_bass()
    res = bass_utils.run_bass_kernel_spmd(
        _NC_CACHE, in_maps, core_ids=list(range(NCORES))
    )
    _LAST_RESULTS = res

    out = np.empty((BS, NJOINTS, NFEATS, NFRAMES), np.float32)
    for c in range(NCORES):
        out[B_PER_CORE * c:B_PER_CORE * (c + 1)] = (
            res.results[c]["out"].reshape(B_PER_CORE, NJOINTS, NFEATS, NFRAMES)
        )
    return out
